# revision 5
# baseline (speedup 1.0000x reference)
"""Causal single-head attention [Sq,B,D]=[2048,4,512] fp32 on 8 TRN2 NeuronCores.

Sharding: core = 2*b + p  (b = batch 0..3, p = query-row parity).
Core (b, p) computes output rows i = 2j + p (j = 0..1023) of batch b.

Key trick for SPMD (one program, 8 cores): queries are strided by 2, and
K/V are host-shifted by s = 1-p rows. Then the causal condition
  k <= i  ==  k' <= 2*j + 1   (k' = shifted key index)
is identical on every core, so the on-device causal mask is a compile-time
affine_select and block extents are core-invariant.

Math per core: S^T[k',j] = K'^T Q^T / sqrt(D) via PE (contract d);
P^T = exp(S^T) (no max subtraction: scores ~ N(0,1), bounded);
causal zeroing via affine_select; O = P V' and r = P @ ones' accumulated
in PSUM over k' chunks; O /= r. Key-mask + shift padding are folded into
V' rows and ones' (zeroed) on the host, so masked keys contribute 0 to
both numerator and denominator. Matmuls run in float32r (full-rate fp32
storage with TF32-like internal rounding).
"""
import math
from contextlib import ExitStack

import numpy as np

import concourse.bass as bass
import concourse.tile as tile
import concourse.mybir as mybir
from concourse import bacc
from concourse.bass_utils import run_bass_kernel_spmd

SQ, SK, B, D = 2048, 2048, 4, 512
N_CORES = 8
QL = SQ // 2          # local q rows per core
QB = 256              # local q-block size
NBLK = QL // QB       # 4 blocks
NKC = SK // 128       # 16 key chunks
EXT = [4 * (m + 1) for m in range(NBLK)]   # k'-chunk extent per block
BAND = 4              # diagonal band width in chunks
SCALE = 1.0 / math.sqrt(D)

_cache = {}


def _build(stage=4, num_devices=N_CORES):
    f32 = mybir.dt.float32
    f32r = mybir.dt.float32r
    Exp = mybir.ActivationFunctionType.Exp

    nc = bacc.Bacc("TRN2", target_bir_lowering=False, debug=False,
                   num_devices=num_devices)
    qt_d = nc.dram_tensor("qt", [D, QL], f32r, kind="ExternalInput").ap()
    kt_d = nc.dram_tensor("kt", [D, SK], f32r, kind="ExternalInput").ap()
    v_d = nc.dram_tensor("v", [SK, D], f32r, kind="ExternalInput").ap()
    ones_d = nc.dram_tensor("ones2d", [128, NKC], f32r, kind="ExternalInput").ap()
    out_d = nc.dram_tensor("out", [QL, D], f32, kind="ExternalOutput").ap()

    with tile.TileContext(nc) as tc, ExitStack() as ctx:
        const = ctx.enter_context(tc.tile_pool(name="const", bufs=1))
        pin = ctx.enter_context(tc.tile_pool(name="pin", bufs=1))
        ppt = ctx.enter_context(tc.tile_pool(name="ppt", bufs=3))
        pst = ctx.enter_context(tc.tile_pool(name="pst", bufs=2, space="PSUM"))
        pacc = ctx.enter_context(tc.tile_pool(name="pacc", bufs=2, space="PSUM"))
        prr = ctx.enter_context(tc.tile_pool(name="prr", bufs=1, space="PSUM"))
        pfin = ctx.enter_context(tc.tile_pool(name="pfin", bufs=2))

        ident = const.tile([1, 1], f32)
        nc.vector.memset(ident[:], 1.0)
        ones_sb = const.tile([128, NKC], f32r)
        nc.sync.dma_start(ones_sb[:], ones_d[:])

        # K^T: 4 d-chunks x 4 column groups of 512 k' (tile-granular deps)
        kt_sb = [[pin.tile([128, 512], f32r, tag=f"kt{dc}_{g}", name=f"kt{dc}_{g}") for g in range(4)]
                 for dc in range(4)]
        for dc in range(4):
            for g in range(4):
                nc.sync.dma_start(kt_sb[dc][g][:],
                                  kt_d[128 * dc:128 * (dc + 1),
                                       512 * g:512 * (g + 1)])
        # Q^T: 4 d-chunks x 4 blocks of QB
        qt_sb = [[pin.tile([128, QB], f32r, tag=f"qt{dc}_{m}", name=f"qt{dc}_{m}") for m in range(NBLK)]
                 for dc in range(4)]
        for dc in range(4):
            for m in range(NBLK):
                nc.sync.dma_start(qt_sb[dc][m][:],
                                  qt_d[128 * dc:128 * (dc + 1),
                                       QB * m:QB * (m + 1)])
        # V chunks [128 k', D]
        v_sb = [pin.tile([128, D], f32r, tag=f"v{c}", name=f"v{c}") for c in range(NKC)]
        for c in range(NKC):
            nc.sync.dma_start(v_sb[c][:], v_d[128 * c:128 * (c + 1), :])

        fill0 = nc.gpsimd.to_reg(0.0)

        for m in range(NBLK):
            ext = EXT[m]
            if stage >= 3:
                o_ps = [pacc.tile([128, D], f32, tag=f"o{j}", name=f"o{m}_{j}") for j in range(2)]
                r_ps = prr.tile([1, QB], f32, tag="r")
            for c in range(ext):
                st = pst.tile([128, QB], f32, tag="st")
                for dc in range(4):
                    nc.tensor.matmul(st[:],
                                     kt_sb[dc][c // 4][:, 128 * (c % 4):128 * (c % 4 + 1)],
                                     qt_sb[dc][m][:],
                                     start=(dc == 0), stop=(dc == 3))
                pt = ppt.tile([128, QB], f32r, tag="pt")
                nc.scalar.activation(pt[:], st[:], Exp, scale=SCALE)
                if stage >= 2 and c >= ext - BAND:
                    # keep where 2*jj - kk + (512*m - 128*c + 1) >= 0
                    nc.gpsimd.affine_select(
                        pt[:], pt[:], pattern=[[2, QB]],
                        compare_op=mybir.AluOpType.is_ge, fill=fill0,
                        base=512 * m - 128 * c + 1, channel_multiplier=-1)
                if stage < 3:
                    if c == ext - 1:
                        dbg = pfin.tile([128, QB], f32, tag="dbg")
                        nc.vector.tensor_copy(dbg[:], pt[:])
                        nc.gpsimd.dma_start(out_d[QB * m:QB * m + 128, 0:QB], dbg[:])
                    continue
                for j in range(2):
                    nc.tensor.matmul(o_ps[j][:],
                                     pt[:, 128 * j:128 * (j + 1)],
                                     v_sb[c][:],
                                     start=(c == 0), stop=(c == ext - 1))
                nc.tensor.matmul(r_ps[:], ones_sb[:, c:c + 1], pt[:],
                                 start=(c == 0), stop=(c == ext - 1))

            if stage < 3:
                continue
            if stage < 4:
                for j in range(2):
                    o_sb = pfin.tile([128, D], f32, tag="osb")
                    nc.vector.tensor_copy(o_sb[:], o_ps[j][:])
                    nc.gpsimd.dma_start(out_d[QB * m + 128 * j:QB * m + 128 * (j + 1), :], o_sb[:])
                continue
            r_sb = pfin.tile([1, QB], f32, tag="rsb")
            nc.scalar.copy(r_sb[:], r_ps[:])
            for j in range(2):
                rt_ps = prr.tile([128, 1], f32, tag="rt")
                nc.tensor.transpose(rt_ps[:], r_sb[0:1, 128 * j:128 * (j + 1)],
                                    ident[:])
                rinv = pfin.tile([128, 1], f32, tag="rinv")
                nc.vector.reciprocal(rinv[:], rt_ps[:])
                o_sb = pfin.tile([128, D], f32, tag="osb")
                nc.vector.tensor_scalar_mul(o_sb[:], o_ps[j][:], rinv[:])
                nc.gpsimd.dma_start(out_d[QB * m + 128 * j:QB * m + 128 * (j + 1), :],
                                    o_sb[:])
    nc.compile()
    return nc


def _prep_core_inputs(Q, K, V, key_mask, b, p):
    s = 1 - p
    qt = np.ascontiguousarray(Q[p::2, b, :].T)            # [D, QL]
    kshift = np.zeros((SK, D), dtype=np.float32)
    vshift = np.zeros((SK, D), dtype=np.float32)
    kshift[s:] = K[:SK - s, b, :]
    vshift[s:] = V[:SK - s, b, :]
    valid = np.zeros(SK, dtype=bool)
    valid[s:] = ~key_mask[:SK - s, b]
    vshift[~valid] = 0.0
    ones2d = valid.astype(np.float32).reshape(NKC, 128).T  # [128, NKC]
    return {
        "qt": qt,
        "kt": np.ascontiguousarray(kshift.T),              # [D, SK]
        "v": vshift,
        "ones2d": np.ascontiguousarray(ones2d),
    }


def run(inputs, trace=False, trace_cores=None):
    if "nc" not in _cache:
        _cache["nc"] = _build()
    nc = _cache["nc"]

    Q = np.asarray(inputs["Q"], dtype=np.float32)
    K = np.asarray(inputs["K"], dtype=np.float32)
    V = np.asarray(inputs["V"], dtype=np.float32)
    key_mask = np.asarray(inputs["key_mask"], dtype=bool)

    in_maps = []
    for core in range(N_CORES):
        b, p = divmod(core, 2)
        in_maps.append(_prep_core_inputs(Q, K, V, key_mask, b, p))

    res = run_bass_kernel_spmd(nc, in_maps, list(range(N_CORES)),
                               trace=trace, trace_cores=trace_cores)

    out = np.empty((SQ, B, D), dtype=np.float32)
    for core in range(N_CORES):
        b, p = divmod(core, 2)
        out[p::2, b, :] = res.results[core]["out"]
    return out, res


def kernel(**inputs):
    out, _ = run(inputs, trace=False)
    return out
